# revision 41
# baseline (speedup 1.0000x reference)
"""BertCrf loss kernel for Trainium2 (8 NeuronCores, SPMD data-parallel).

Strategy
--------
Shapes: B=64, S=512, H=768, T=9 tags.  Loss = -sum_b(num_b - den_b).

The only heavy data is hidden_states [64,512,768] f32 (100 MB) -> the kernel
is memory-bound on streaming it once.  Each of the 8 cores takes 8 sequences.

Phase 1 (device, DMA-bound): emissions via fp8 DoubleRow matmuls into 8 PSUM
banks of [16, 512] (bank g = token cols [512g, 512g+512), t-major).  The
stream is 12 column-quarter DMAs (ktile-pair x quarter) on the two HWDGE
rings; quarter order is banks (1,2),(3,4),(5,6),(7,0) -- see below.

The numerator needs only e[b,t,label[b,t]], which the host computes directly
from hidden_states and fc_w (a gather + dot, ~50 M MAC) -- so nothing of e
ever leaves the device and the old eT evacuation path is gone entirely.

Phase 2 (device): the CRF log-partition recurrence in linear space
  P <- P @ (E * f_t[None,:]),  E = exp(trans), f_t = exp(e_t + fc_b - sigma)
as 512 independent 9x9 chunk-matrix chains (8 seqs x 64 chunks of L=8
steps), advanced by 8 steps of one [72,72]x[72,288] bf16 matmul + one
[72,288] VectorE scale per half-chain (halves ping-pong on PE/DVE).

F production (the old kernel's bottleneck -- it cost a ~9us stall): each
PSUM bank is evacuated by ONE fused ACT op (exp + per-tag bias, PSUM input)
-- no separate copy, no eT staging.  DoubleRow matmuls can only write PSUM
partitions [0,16), and ACT/DVE are lane-locked, so group g's F rows [9g,9g+9)
need a +9g partition shift: exp lands in a staging tile column slot, and a
small SBUF->SBUF fold DMA (9 descriptors, SWDGE) issues immediately after.
Group 0 needs no shift, so it streams LAST and exps directly into f_all;
groups 1-7 stream earlier so their folds hide under the remaining stream.
Only the bank-7 fold (~1us) plus group-0's exp trail the last matmul.

DMA: 3.15MB/core of hidden cast to fp8-e4m3 on host (loss insensitive:
emission errors largely cancel between numerator and denominator; measured
~1e-4 end-to-end).  Constants are host-prepacked (fc weights in the exact
DoubleRow SBUF image) and loaded via SWDGE, independent of the stream rings.

Host (cheap, exact f64): numerator from raw inputs; combine the 64 chunk
matrices per sequence (tiny 9x9 matvecs, vectorized over sequences) with
renormalization; final logsumexp with end_transitions.  A full numpy
fallback handles any non-all-ones attention mask.
"""

import numpy as np

# ---- problem constants (hardcoded per the task contract) ----
B, S, H, T = 64, 512, 768, 9
NCORES = 8
NB = B // NCORES          # 8 local sequences per core
NTOK = NB * S             # 4096 tokens per core
L = 8                     # chunk length (timesteps per chunk)
C = S // L                # 64 chunks
NG = 8                    # partition groups; chunk c = 8g + c3
C3 = 8                    # chunks per group
P_SCAN = 128              # scan partition space (sparse rows, junk zeroed)
P_OUT = 114               # rows actually carrying data (see _row)
NFREE = C3 * NB * T       # 576 scan free columns (c3, b, i)
HALF = NFREE // 2         # 288 free columns per scan half-chain
SIGMA = 0.8               # linear-space shift (range control)
KT = H // 128             # 6 contraction tiles
# stream slot -> banks; DMA-fold banks early, single-bank tail slots so the
# final folds and bank 0's exp overlap the end of the stream
SLOT_BANKS = [(1, 2), (3, 4), (5, 6), (7,), (0,)]


# Scan-row map.  Group 0 exps directly into rows [0,9); groups 1,2 DMA-fold
# into [9,27) (early-streamed, latency hidden); groups 3-7 fold via DVE
# stream_shuffles into the 32-aligned windows [32:64), [64:96), [96:128)
# (two groups per window via a chained shuffle; window junk rows are dead).
# Chained pairs are (3,4) early-stream and (6,7) where bank 7's exp writes
# the ftmp2 slot directly, so the last fold is a single DVE hop.
_ROWS = {0: 0, 1: 9, 2: 18, 3: 41, 4: 32, 5: 64, 6: 105, 7: 96}


def _row(g):
    return _ROWS[g]

# token order is t-major: column index = t*NB + b.  Bank g of the emissions
# matmul (token cols [512g, 512g+512)) is exactly scan group g's (c3, s, b)
# panel: col = 512g + 64*c3 + 8*s + b.

_cached = {}


def _np_logsumexp(x, axis):
    m = np.max(x, axis=axis, keepdims=True)
    return (m + np.log(np.sum(np.exp(x - m), axis=axis, keepdims=True))).squeeze(axis)


def _reference_host(hidden_states, attention_mask, labels, fc_w, fc_b,
                    start_transitions, end_transitions, transitions):
    """Exact numpy port of the reference (f64) - fallback for unusual inputs."""
    e = (hidden_states.astype(np.float64) @ fc_w.T.astype(np.float64)) + fc_b
    mask = attention_mask.astype(bool)
    maskf = mask.astype(np.float64)
    labels = labels.astype(np.int64)
    b_idx = np.arange(e.shape[0])

    emit = np.take_along_axis(e, labels[..., None], axis=-1)[..., 0]
    trans_sc = transitions[labels[:, :-1], labels[:, 1:]].astype(np.float64)
    num = start_transitions[labels[:, 0]].astype(np.float64) + emit[:, 0]
    num = num + ((trans_sc + emit[:, 1:]) * maskf[:, 1:]).sum(1)
    last_idx = mask.astype(np.int64).sum(1) - 1
    num = num + end_transitions[labels[b_idx, last_idx]]

    alpha = start_transitions[None, :].astype(np.float64) + e[:, 0]
    for t in range(1, e.shape[1]):
        nxt = _np_logsumexp(alpha[:, :, None] + transitions[None].astype(np.float64)
                            + e[:, t][:, None, :], axis=1)
        alpha = np.where(mask[:, t][:, None], nxt, alpha)
    den = _np_logsumexp(alpha + end_transitions[None, :].astype(np.float64), axis=1)
    return np.float32(-(num - den).sum())


def _build_nc():
    """Build the per-core Bass program (same program on all 8 cores)."""
    import concourse.bacc as bacc
    import concourse.mybir as mybir
    import concourse.tile as tile

    dt = mybir.dt
    mdt = dt.float8e4

    # Bacc (not raw Bass): its compile() pass legalizes multi-wait sync_info
    # into what this walrus build's per-instruction wait slots accept.
    nc = bacc.Bacc("TRN2", target_bir_lowering=False, debug=False)

    # stream: one DMA per (slot, ktile-pair), laid out back to back in one
    # flat DRAM buffer [128, 6*4096]; within a tile, partition p holds
    # ktiles (2*ktp, 2*ktp+1) side by side for DoubleRow.
    hTq = nc.dram_tensor("hTq", [128, KT * NTOK], mdt, kind="ExternalInput")
    # fc weights pre-packed in the exact DoubleRow SBUF image (9->16 pad baked)
    fcwp = nc.dram_tensor("fcwp", [128, KT // 2, 2, 16], mdt, kind="ExternalInput")
    lhsE = nc.dram_tensor("lhsE", [P_SCAN, P_SCAN], dt.bfloat16, kind="ExternalInput")
    epat = nc.dram_tensor("epat", [P_SCAN, NFREE], dt.float32, kind="ExternalInput")
    biasF = nc.dram_tensor("biasF", [T, 1], dt.float32, kind="ExternalInput")
    q_out = nc.dram_tensor("q_out", [P_OUT, NFREE], dt.bfloat16,
                           kind="ExternalOutput")

    with tile.TileContext(nc) as tc:
        with (
            tc.tile_pool(name="const", bufs=1) as cpool,
            tc.tile_pool(name="hbuf", bufs=1) as hpool,
            tc.tile_pool(name="fbuf", bufs=1) as fpool,
            tc.tile_pool(name="scan", bufs=2) as qpool,
        ):
            # ---- constants on the SWDGE path (independent of the two
            # HWDGE stream rings); all host-prepacked into clean shapes ----
            fcw_sb = cpool.tile([128, KT // 2, 2, 16], mdt)
            nc.gpsimd.dma_start(fcw_sb, fcwp[:, :, :, :])
            biasF_sb = cpool.tile([T, 1], dt.float32)
            nc.gpsimd.dma_start(biasF_sb, biasF[:, :])
            lhsE_sb = cpool.tile([P_SCAN, P_SCAN], dt.bfloat16)
            nc.gpsimd.dma_start(lhsE_sb, lhsE[:, :])
            epat_sb = cpool.tile([P_SCAN, NFREE], dt.float32)
            nc.gpsimd.dma_start(epat_sb, epat[:, :])

            # F in scan layout [_row(g)+j, (c3,s,b)] plus staging tiles:
            # ftmp column slot per group (exp output, bank partitions [0,9)),
            # ftmp2 one slot per chained-shuffle window.  Junk rows must be
            # finite: psq junk rows are exact zeros via lhsE/epat zeros, but
            # 0 * inf would poison the contraction, so memset everything
            # (vector engine, idle early).
            f_all = fpool.tile([P_SCAN, C3 * L * NB], dt.float32)
            nc.vector.memset(f_all, 0.0)
            ftmp = fpool.tile([32, 5 * 512], dt.float32)
            nc.vector.memset(ftmp, 0.0)
            ftmp2 = fpool.tile([32, 2 * 512], dt.float32)
            FSLOT = {1: 0, 2: 1, 3: 2, 5: 3, 6: 4}   # ftmp column slots
            WSLOT = {3: 0, 4: 0, 6: 1, 7: 1}         # chained-window slots
            WBASE = {4: 32, 5: 64, 7: 96}            # shuffle window bases

            # ---- stream DMAs, all issued from sync: the scalar engine must
            # stay free for the exp chain (it is the only engine with Exp,
            # and the exps are on the critical path) ----
            htiles = {}
            off = 0
            for slot, banks in enumerate(SLOT_BANKS):
                w = len(banks)
                for ktp in range(KT // 2):
                    ht = hpool.tile([128, 2, 512 * w], mdt,
                                    tag=f"ht{slot}_{ktp}", name=f"ht{slot}_{ktp}")
                    nc.sync.dma_start(
                        ht, hTq[:, off:off + 1024 * w]
                        .rearrange("p (two c) -> p two c", two=2))
                    off += 1024 * w
                    htiles[(slot, ktp)] = ht

            with tc.tile_pool(name="psum1", bufs=1, space="PSUM") as pspool:
                psbank = [pspool.tile([16, 512], dt.float32, tag=f"psb{g}",
                                      name=f"psb{g}")
                          for g in range(NG)]
                for slot, banks in enumerate(SLOT_BANKS):
                    for bk, g in enumerate(banks):
                        for ktp in range(KT // 2):
                            nc.tensor.matmul(
                                psbank[g],
                                fcw_sb[:, ktp],
                                htiles[(slot, ktp)][:, :, bk * 512:(bk + 1) * 512],
                                start=(ktp == 0), stop=(ktp == KT // 2 - 1),
                                perf_mode=mybir.MatmulPerfMode.DoubleRow)
                        # fused PSUM->F exp with per-tag bias (fc_b - sigma).
                        # chunk 0's start-transition correction lives in
                        # epat (host bakes exp(start_j + sigma) there).
                        if g == 0:
                            nc.scalar.activation(
                                f_all[0:T, :], psbank[0][0:T, :],
                                mybir.ActivationFunctionType.Exp,
                                bias=biasF_sb)
                        elif g in (4, 7):
                            # second group of a chained window: exp straight
                            # into ftmp2 lanes [0,9) (shuffle1 of the first
                            # group already parked lanes [9,18)), then one
                            # shuffle moves all 18 lanes into the window
                            w = WSLOT[g]
                            nc.scalar.activation(
                                ftmp2[0:T, w * 512:(w + 1) * 512],
                                psbank[g][0:T, :],
                                mybir.ActivationFunctionType.Exp,
                                bias=biasF_sb)
                            mask = [i if i < 2 * T else 0 for i in range(32)]
                            nc.vector.stream_shuffle(
                                f_all[WBASE[g]:WBASE[g] + 32, :],
                                ftmp2[0:32, w * 512:(w + 1) * 512], mask)
                        else:
                            fs = FSLOT[g]
                            dst = ftmp[0:T, fs * 512:(fs + 1) * 512]
                            nc.scalar.activation(
                                dst, psbank[g][0:T, :],
                                mybir.ActivationFunctionType.Exp,
                                bias=biasF_sb)
                            if g in (1, 2):
                                # fold +9g partitions via SWDGE while later
                                # banks stream on the HWDGE rings
                                nc.gpsimd.dma_start(
                                    f_all[T * g:T * g + T, :], dst)
                            elif g == 5:
                                # single-group window [64:96)
                                mask = [i if i < T else 0
                                        for i in range(32)]
                                nc.vector.stream_shuffle(
                                    f_all[64:96, :],
                                    ftmp[0:32, fs * 512:(fs + 1) * 512],
                                    mask)
                            else:
                                # first group of a chained window: park its
                                # lanes at [9,18) of the ftmp2 slot
                                w = WSLOT[g]
                                mask = [i - T if T <= i < 2 * T else 0
                                        for i in range(32)]
                                nc.vector.stream_shuffle(
                                    ftmp2[0:32, w * 512:(w + 1) * 512],
                                    ftmp[0:32, fs * 512:(fs + 1) * 512],
                                    mask)

            # F_all[(g,j), (c3, s, b)] ready.  Scan: chunk c = 8g + c3,
            # 8 steps, two half-chains (c3 0-3 | 4-7) interleaved so PE and
            # DVE ping-pong.
            f_v = f_all.rearrange("p (c3 s b) -> p c3 s b", c3=C3, s=L)

            def fslice(s, h):
                return f_v[:, 4 * h:4 * h + 4, s, :].unsqueeze(-1).broadcast_to(
                    [P_SCAN, 4, NB, T])

            with tc.tile_pool(name="psq", bufs=4, space="PSUM") as psqpool:
                qcur = []
                for h in range(2):
                    q = qpool.tile([P_SCAN, 4, NB, T], dt.bfloat16, tag=f"q{h}",
                                   name=f"q{h}")
                    ep = epat_sb[:, h * HALF:(h + 1) * HALF]
                    nc.vector.tensor_mul(
                        q, ep.rearrange("p (c3 b i) -> p c3 b i", c3=4, b=NB),
                        fslice(0, h))
                    qcur.append(q)
                for s in range(1, L):
                    for h in range(2):
                        psq = psqpool.tile([P_SCAN, HALF], dt.float32, tag="psq",
                                           name="psq")
                        nc.tensor.matmul(
                            psq, lhsE_sb,
                            qcur[h].rearrange("p c3 b i -> p (c3 b i)"),
                            start=True, stop=True)
                        qn = qpool.tile([P_SCAN, 4, NB, T], dt.bfloat16,
                                        tag=f"qf{h}" if s == L - 1 else f"q{h}",
                                        name=f"qn{h}_{s}")
                        nc.vector.tensor_mul(
                            qn, psq.rearrange("p (c3 b i) -> p c3 b i",
                                              c3=4, b=NB),
                            fslice(s, h))
                        qcur[h] = qn
                for h in range(2):
                    eng = nc.sync if h == 0 else nc.scalar
                    eng.dma_start(
                        q_out[:, h * HALF:(h + 1) * HALF],
                        qcur[h].rearrange("p c3 b i -> p (c3 b i)")[0:P_OUT])

    nc.compile()
    return nc


def _get_nc():
    if "nc" not in _cached:
        _cached["nc"] = _build_nc()
    return _cached["nc"]


def _host_prep(hidden_states, fc_w, fc_b, start_transitions, transitions):
    """Build the 8 per-core input maps."""
    import ml_dtypes
    np_mdt = ml_dtypes.float8_e4m3

    E = np.exp(transitions.astype(np.float64)).astype(np.float32)     # [T,T]
    # epat[_row(g)+j,(c3,b,i)] = E[i,j], except chunk 0 (g=0, c3=0) slots =
    # exp(start_j + sigma) so the step-0 multiply by F = exp(e + fcb - sigma)
    # lands exactly on alpha_0 = exp(start + fcb + e_0) with no fixup op;
    # junk rows stay 0 so junk lanes propagate exact zeros
    epat = np.zeros((P_SCAN, C3, NB, T), dtype=np.float32)
    blk = np.tile(E.T[:, None, None, :], (1, C3, NB, 1))              # [j,c3,b,i]
    for g in range(NG):
        epat[_row(g):_row(g) + T] = blk
    epat[0:T, 0, :, :] = np.exp(
        start_transitions.astype(np.float64) + SIGMA)[:, None, None]
    epat = np.ascontiguousarray(epat.reshape(P_SCAN, NFREE))
    # lhsE: lhsT[_row(g)+k, _row(g)+j] = E[k,j]  (bf16 scan matmul)
    lhsE = np.zeros((P_SCAN, P_SCAN), dtype=ml_dtypes.bfloat16)
    Eb = E.astype(ml_dtypes.bfloat16)
    for g in range(NG):
        r = _row(g)
        lhsE[r:r + T, r:r + T] = Eb
    # fc weights in the DoubleRow SBUF image [128, ktp, two, 16], 9->16 pad
    fcwp = np.zeros((128, KT // 2, 2, 16), dtype=np_mdt)
    w = fc_w.astype(np_mdt)                                           # [T, H]
    for ktp in range(KT // 2):
        for two in range(2):
            kt = 2 * ktp + two
            fcwp[:, ktp, two, 0:T] = w[:, kt * 128:(kt + 1) * 128].T
    biasF = np.ascontiguousarray(
        (fc_b - SIGMA).reshape(T, 1), dtype=np.float32)

    in_maps = []
    for cid in range(NCORES):
        hc = hidden_states[cid * NB:(cid + 1) * NB]                   # [NB,S,H]
        # t-major token order: col = t*NB + b
        hc = hc.transpose(1, 0, 2).reshape(NTOK, H)
        hT6 = np.ascontiguousarray(hc.T.astype(np_mdt)).reshape(KT, 128, NTOK)
        hTq = np.empty((128, KT * NTOK), dtype=np_mdt)
        off = 0
        for banks in SLOT_BANKS:
            for ktp in range(KT // 2):
                for two in range(2):
                    kt = 2 * ktp + two
                    for g in banks:
                        hTq[:, off:off + 512] = hT6[kt, :, 512 * g:512 * g + 512]
                        off += 512
        in_maps.append({
            "hTq": hTq, "fcwp": fcwp, "lhsE": lhsE, "epat": epat,
            "biasF": biasF,
        })
    return in_maps


def _host_finish(results, hidden_states, labels, fc_w, fc_b,
                 start_transitions, end_transitions, transitions):
    """Numerator from raw inputs + chunk-matrix combine, all in f64."""
    labels = labels.astype(np.int64)
    start = start_transitions.astype(np.float64)
    end = end_transitions.astype(np.float64)
    trans = transitions.astype(np.float64)

    # numerator (mask all-ones fast path): emit[b,t] = h[b,t] . fc_w[l[b,t]]
    wg = fc_w[labels]                                     # [B,S,H] f32
    emit = np.einsum('bsh,bsh->bs', hidden_states, wg,
                     dtype=np.float64) + fc_b[labels].astype(np.float64)
    num = start[labels[:, 0]] + emit[:, 0]
    num = num + (trans[labels[:, :-1], labels[:, 1:]] + emit[:, 1:]).sum(1)
    num = num + end[labels[:, -1]]

    # denominator: combine chunk matrices.  q_out[_row(g)+j, (c3,b,i)] =
    # P_c[i,j] with c = 8g + c3.  Vectorize the chain over all sequences.
    P = np.empty((B, C, T, T))                            # P[b,c,i,j]
    for cid in range(NCORES):
        Q = results[cid]["q_out"].astype(np.float64)      # [105, 576]
        Qr = Q.reshape(P_OUT, C3, NB, T)                  # [p, c3, b, i]
        for g in range(NG):
            r = _row(g)
            # [j, c3, b, i] -> [b, c3, i, j]
            P[cid * NB:(cid + 1) * NB, 8 * g:8 * g + 8] = \
                Qr[r:r + T].transpose(2, 1, 3, 0)
    alpha = P[:, 0, 0, :].copy()          # rows of P_0 all equal alpha_0
    corr = np.zeros(B)
    for c in range(1, C):
        alpha = np.einsum('bi,bij->bj', alpha, P[:, c])
        m = alpha.max(axis=1)
        alpha /= m[:, None]
        corr += np.log(m)
    den = np.log((alpha * np.exp(end)[None, :]).sum(1)) + corr + (S - 1) * SIGMA
    return np.float32(-(num - den).sum())


def kernel(**inputs):
    hidden_states = np.asarray(inputs["hidden_states"], dtype=np.float32)
    attention_mask = np.asarray(inputs["attention_mask"])
    labels = np.asarray(inputs["labels"])
    fc_w = np.asarray(inputs["fc_w"], dtype=np.float32)
    fc_b = np.asarray(inputs["fc_b"], dtype=np.float32)
    start_transitions = np.asarray(inputs["start_transitions"], dtype=np.float32)
    end_transitions = np.asarray(inputs["end_transitions"], dtype=np.float32)
    transitions = np.asarray(inputs["transitions"], dtype=np.float32)

    if (hidden_states.shape != (B, S, H)) or not np.all(attention_mask != 0):
        return _reference_host(hidden_states, attention_mask, labels, fc_w,
                               fc_b, start_transitions, end_transitions,
                               transitions)

    from concourse.bass_utils import run_bass_kernel_spmd
    nc = _get_nc()
    in_maps = _host_prep(hidden_states, fc_w, fc_b, start_transitions,
                         transitions)
    res = run_bass_kernel_spmd(nc, in_maps, core_ids=list(range(NCORES)))
    _cached["last_res"] = res
    return _host_finish(res.results, hidden_states, labels, fc_w, fc_b,
                        start_transitions, end_transitions, transitions)


# revision 47
# speedup vs baseline: 1.0557x; 1.0557x over previous
"""BertCrf loss kernel for Trainium2 (8 NeuronCores, SPMD data-parallel).

Strategy
--------
Shapes: B=64, S=512, H=768, T=9 tags.  Loss = -sum_b(num_b - den_b).

The only heavy data is hidden_states [64,512,768] f32 (100 MB) -> the kernel
is memory-bound on streaming it once.  Each of the 8 cores takes 8 sequences.

Phase 1 (device, DMA-bound): emissions via fp8 DoubleRow matmuls into 8 PSUM
banks of [16, 512] (bank g = token cols [512g, 512g+512), t-major).  The
stream is 12 column-quarter DMAs (ktile-pair x quarter) on the two HWDGE
rings; quarter order is banks (1,2),(3,4),(5,6),(7,0) -- see below.

The numerator needs only e[b,t,label[b,t]], which the host computes directly
from hidden_states and fc_w (a gather + dot, ~50 M MAC) -- so nothing of e
ever leaves the device and the old eT evacuation path is gone entirely.

Phase 2 (device): the CRF log-partition recurrence in linear space
  P <- P @ (E * f_t[None,:]),  E = exp(trans), f_t = exp(e_t + fc_b - sigma)
as 512 independent 9x9 chunk-matrix chains (8 seqs x 64 chunks of L=8
steps), advanced by 8 steps of one [72,72]x[72,288] bf16 matmul + one
[72,288] VectorE scale per half-chain (halves ping-pong on PE/DVE).

F production (the old kernel's bottleneck -- it cost a ~9us stall): each
PSUM bank is evacuated by ONE fused ACT op (exp + per-tag bias, PSUM input)
-- no separate copy, no eT staging.  DoubleRow matmuls can only write PSUM
partitions [0,16), and ACT/DVE are lane-locked, so group g's F rows [9g,9g+9)
need a +9g partition shift: exp lands in a staging tile column slot, and a
small SBUF->SBUF fold DMA (9 descriptors, SWDGE) issues immediately after.
Group 0 needs no shift, so it streams LAST and exps directly into f_all;
groups 1-7 stream earlier so their folds hide under the remaining stream.
Only the bank-7 fold (~1us) plus group-0's exp trail the last matmul.

DMA: 3.15MB/core of hidden cast to fp8-e4m3 on host (loss insensitive:
emission errors largely cancel between numerator and denominator; measured
~1e-4 end-to-end).  Constants are host-prepacked (fc weights in the exact
DoubleRow SBUF image) and loaded via SWDGE, independent of the stream rings.

Host (cheap, exact f64): numerator from raw inputs; combine the 64 chunk
matrices per sequence (tiny 9x9 matvecs, vectorized over sequences) with
renormalization; final logsumexp with end_transitions.  A full numpy
fallback handles any non-all-ones attention mask.
"""

import numpy as np

# ---- problem constants (hardcoded per the task contract) ----
B, S, H, T = 64, 512, 768, 9
NCORES = 8
NB = B // NCORES          # 8 local sequences per core
NTOK = NB * S             # 4096 tokens per core
L = 8                     # chunk length (timesteps per chunk)
C = S // L                # 64 chunks
NG = 8                    # partition groups; chunk c = 8g + c3
C3 = 8                    # chunks per group
P_SCAN = 128              # scan partition space (sparse rows, junk zeroed)
P_OUT = 105               # rows actually carrying data (see _row)
NFREE = C3 * NB * T       # 576 scan free columns (c3, b, i)
HALF = NFREE // 2         # 288 free columns per scan half-chain
SIGMA = 0.8               # linear-space shift (range control)
KT = H // 128             # 6 contraction tiles
# stream slot -> banks; DMA-fold banks early, single-bank tail slots so the
# final folds and bank 0's exp overlap the end of the stream
SLOT_BANKS = [(1, 2), (3, 4), (5, 6), (7,), (0,)]


# Scan-row map.  Group 0 exps directly into rows [0,9); groups 1,2 DMA-fold
# into [9,27) (early-streamed, latency hidden); groups 3-7 fold via DVE
# stream_shuffles into the 32-aligned windows [32:64), [64:96), [96:128)
# (two groups per window via a chained shuffle; window junk rows are dead).
# Chained pairs are (3,4) and (5,6), both early enough to hide their serial
# exp->park->exp->shuffle hops; bank 7 is a single DVE hop at the end.
_ROWS = {0: 0, 1: 9, 2: 18, 3: 41, 4: 32, 5: 73, 6: 64, 7: 96}


def _row(g):
    return _ROWS[g]

# token order is t-major: column index = t*NB + b.  Bank g of the emissions
# matmul (token cols [512g, 512g+512)) is exactly scan group g's (c3, s, b)
# panel: col = 512g + 64*c3 + 8*s + b.

_cached = {}


def _np_logsumexp(x, axis):
    m = np.max(x, axis=axis, keepdims=True)
    return (m + np.log(np.sum(np.exp(x - m), axis=axis, keepdims=True))).squeeze(axis)


def _reference_host(hidden_states, attention_mask, labels, fc_w, fc_b,
                    start_transitions, end_transitions, transitions):
    """Exact numpy port of the reference (f64) - fallback for unusual inputs."""
    e = (hidden_states.astype(np.float64) @ fc_w.T.astype(np.float64)) + fc_b
    mask = attention_mask.astype(bool)
    maskf = mask.astype(np.float64)
    labels = labels.astype(np.int64)
    b_idx = np.arange(e.shape[0])

    emit = np.take_along_axis(e, labels[..., None], axis=-1)[..., 0]
    trans_sc = transitions[labels[:, :-1], labels[:, 1:]].astype(np.float64)
    num = start_transitions[labels[:, 0]].astype(np.float64) + emit[:, 0]
    num = num + ((trans_sc + emit[:, 1:]) * maskf[:, 1:]).sum(1)
    last_idx = mask.astype(np.int64).sum(1) - 1
    num = num + end_transitions[labels[b_idx, last_idx]]

    alpha = start_transitions[None, :].astype(np.float64) + e[:, 0]
    for t in range(1, e.shape[1]):
        nxt = _np_logsumexp(alpha[:, :, None] + transitions[None].astype(np.float64)
                            + e[:, t][:, None, :], axis=1)
        alpha = np.where(mask[:, t][:, None], nxt, alpha)
    den = _np_logsumexp(alpha + end_transitions[None, :].astype(np.float64), axis=1)
    return np.float32(-(num - den).sum())


def _build_nc():
    """Build the per-core Bass program (same program on all 8 cores)."""
    import concourse.bacc as bacc
    import concourse.mybir as mybir
    import concourse.tile as tile

    dt = mybir.dt
    mdt = dt.float8e4

    # Bacc (not raw Bass): its compile() pass legalizes multi-wait sync_info
    # into what this walrus build's per-instruction wait slots accept.
    nc = bacc.Bacc("TRN2", target_bir_lowering=False, debug=False)

    # stream: one DMA per (slot, ktile-pair), laid out back to back in one
    # flat DRAM buffer [128, 6*4096]; within a tile, partition p holds
    # ktiles (2*ktp, 2*ktp+1) side by side for DoubleRow.
    hTq = nc.dram_tensor("hTq", [128, KT * NTOK], mdt, kind="ExternalInput")
    # fc weights pre-packed in the exact DoubleRow SBUF image (9->16 pad baked)
    fcwp = nc.dram_tensor("fcwp", [128, KT // 2, 2, 16], mdt, kind="ExternalInput")
    lhsE = nc.dram_tensor("lhsE", [P_SCAN, P_SCAN], dt.bfloat16, kind="ExternalInput")
    epat = nc.dram_tensor("epat", [P_SCAN, NFREE], dt.float32, kind="ExternalInput")
    biasF = nc.dram_tensor("biasF", [T, 1], dt.float32, kind="ExternalInput")
    q_out = nc.dram_tensor("q_out", [P_OUT, NFREE], dt.bfloat16,
                           kind="ExternalOutput")

    with tile.TileContext(nc) as tc:
        with (
            tc.tile_pool(name="const", bufs=1) as cpool,
            tc.tile_pool(name="hbuf", bufs=1) as hpool,
            tc.tile_pool(name="fbuf", bufs=1) as fpool,
            tc.tile_pool(name="scan", bufs=2) as qpool,
        ):
            # ---- constants on the SWDGE path (independent of the two
            # HWDGE stream rings); all host-prepacked into clean shapes ----
            fcw_sb = cpool.tile([128, KT // 2, 2, 16], mdt)
            nc.gpsimd.dma_start(fcw_sb, fcwp[:, :, :, :])
            biasF_sb = cpool.tile([T, 1], dt.float32)
            nc.gpsimd.dma_start(biasF_sb, biasF[:, :])
            lhsE_sb = cpool.tile([P_SCAN, P_SCAN], dt.bfloat16)
            nc.gpsimd.dma_start(lhsE_sb, lhsE[:, :])
            epat_sb = cpool.tile([P_SCAN, NFREE], dt.float32)
            nc.gpsimd.dma_start(epat_sb, epat[:, :])

            # F in scan layout [_row(g)+j, (c3,s,b)] plus staging tiles:
            # ftmp column slot per group (exp output, bank partitions [0,9)),
            # ftmp2 one slot per chained-shuffle window.  Junk rows read by
            # the scan must be finite (0 * inf poisons the contraction even
            # though lhsE/epat zeros keep real lanes exact): the shuffle
            # masks only ever read written lanes, so the one hole is f_all
            # rows [27,32), zeroed by a tiny memset.  No other memsets -- a
            # large one here gates ACT_TABLE_LOAD and delays every exp.
            f_all = fpool.tile([P_SCAN, C3 * L * NB], dt.float32)
            nc.gpsimd.memset(f_all[0:32, :], 0.0)
            ftmp = fpool.tile([32, 5 * 512], dt.float32)
            ftmp2 = fpool.tile([32, 2 * 512], dt.float32)
            FSLOT = {1: 0, 2: 1, 3: 2, 5: 3, 7: 4}   # ftmp column slots
            WSLOT = {3: 0, 4: 0, 5: 1, 6: 1}         # chained-window slots
            WBASE = {4: 32, 6: 64, 7: 96}            # shuffle window bases

            # ---- stream DMAs, all issued from sync: the scalar engine must
            # stay free for the exp chain (it is the only engine with Exp,
            # and the exps are on the critical path) ----
            htiles = {}
            off = 0
            for slot, banks in enumerate(SLOT_BANKS):
                w = len(banks)
                for ktp in range(KT // 2):
                    ht = hpool.tile([128, 2, 512 * w], mdt,
                                    tag=f"ht{slot}_{ktp}", name=f"ht{slot}_{ktp}")
                    nc.sync.dma_start(
                        ht, hTq[:, off:off + 1024 * w]
                        .rearrange("p (two c) -> p two c", two=2))
                    off += 1024 * w
                    htiles[(slot, ktp)] = ht

            with tc.tile_pool(name="psum1", bufs=1, space="PSUM") as pspool:
                psbank = [pspool.tile([16, 512], dt.float32, tag=f"psb{g}",
                                      name=f"psb{g}")
                          for g in range(NG)]
                for slot, banks in enumerate(SLOT_BANKS):
                    for bk, g in enumerate(banks):
                        for ktp in range(KT // 2):
                            nc.tensor.matmul(
                                psbank[g],
                                fcw_sb[:, ktp],
                                htiles[(slot, ktp)][:, :, bk * 512:(bk + 1) * 512],
                                start=(ktp == 0), stop=(ktp == KT // 2 - 1),
                                perf_mode=mybir.MatmulPerfMode.DoubleRow)
                        # fused PSUM->F exp with per-tag bias (fc_b - sigma).
                        # chunk 0's start-transition correction lives in
                        # epat (host bakes exp(start_j + sigma) there).
                        if g == 0:
                            nc.scalar.activation(
                                f_all[0:T, :], psbank[0][0:T, :],
                                mybir.ActivationFunctionType.Exp,
                                bias=biasF_sb)
                        elif g in (4, 6):
                            # second group of a chained window: exp straight
                            # into ftmp2 lanes [0,9) (shuffle1 of the first
                            # group already parked lanes [9,18)), then one
                            # shuffle moves all 18 lanes into the window
                            w = WSLOT[g]
                            nc.scalar.activation(
                                ftmp2[0:T, w * 512:(w + 1) * 512],
                                psbank[g][0:T, :],
                                mybir.ActivationFunctionType.Exp,
                                bias=biasF_sb)
                            mask = [i if i < 2 * T else 0 for i in range(32)]
                            nc.vector.stream_shuffle(
                                f_all[WBASE[g]:WBASE[g] + 32, :],
                                ftmp2[0:32, w * 512:(w + 1) * 512], mask)
                        else:
                            fs = FSLOT[g]
                            dst = ftmp[0:T, fs * 512:(fs + 1) * 512]
                            nc.scalar.activation(
                                dst, psbank[g][0:T, :],
                                mybir.ActivationFunctionType.Exp,
                                bias=biasF_sb)
                            if g in (1, 2):
                                # fold +9g partitions via SWDGE while later
                                # banks stream on the HWDGE rings
                                nc.gpsimd.dma_start(
                                    f_all[T * g:T * g + T, :], dst)
                            elif g == 7:
                                # single-group window [96:128)
                                mask = [i if i < T else 0
                                        for i in range(32)]
                                nc.vector.stream_shuffle(
                                    f_all[96:128, :],
                                    ftmp[0:32, fs * 512:(fs + 1) * 512],
                                    mask)
                            else:
                                # first group of a chained window: park its
                                # lanes at [9,18) of the ftmp2 slot
                                w = WSLOT[g]
                                mask = [i - T if T <= i < 2 * T else 0
                                        for i in range(32)]
                                nc.vector.stream_shuffle(
                                    ftmp2[0:32, w * 512:(w + 1) * 512],
                                    ftmp[0:32, fs * 512:(fs + 1) * 512],
                                    mask)

            # F_all[(g,j), (c3, s, b)] ready.  Scan: chunk c = 8g + c3,
            # 8 steps, two half-chains (c3 0-3 | 4-7) interleaved so PE and
            # DVE ping-pong.
            f_v = f_all.rearrange("p (c3 s b) -> p c3 s b", c3=C3, s=L)

            def fslice(s, h):
                return f_v[:, 4 * h:4 * h + 4, s, :].unsqueeze(-1).broadcast_to(
                    [P_SCAN, 4, NB, T])

            with tc.tile_pool(name="psq", bufs=4, space="PSUM") as psqpool:
                qcur = []
                for h in range(2):
                    q = qpool.tile([P_SCAN, 4, NB, T], dt.bfloat16, tag=f"q{h}",
                                   name=f"q{h}")
                    ep = epat_sb[:, h * HALF:(h + 1) * HALF]
                    nc.vector.tensor_mul(
                        q, ep.rearrange("p (c3 b i) -> p c3 b i", c3=4, b=NB),
                        fslice(0, h))
                    qcur.append(q)
                for s in range(1, L):
                    for h in range(2):
                        psq = psqpool.tile([P_SCAN, HALF], dt.float32, tag="psq",
                                           name="psq")
                        nc.tensor.matmul(
                            psq, lhsE_sb,
                            qcur[h].rearrange("p c3 b i -> p (c3 b i)"),
                            start=True, stop=True)
                        qn = qpool.tile([P_SCAN, 4, NB, T], dt.bfloat16,
                                        tag=f"qf{h}" if s == L - 1 else f"q{h}",
                                        name=f"qn{h}_{s}")
                        nc.vector.tensor_mul(
                            qn, psq.rearrange("p (c3 b i) -> p c3 b i",
                                              c3=4, b=NB),
                            fslice(s, h))
                        qcur[h] = qn
                for h in range(2):
                    eng = nc.sync if h == 0 else nc.scalar
                    eng.dma_start(
                        q_out[:, h * HALF:(h + 1) * HALF],
                        qcur[h].rearrange("p c3 b i -> p (c3 b i)")[0:P_OUT])

    nc.compile()
    return nc


def _get_nc():
    if "nc" not in _cached:
        _cached["nc"] = _build_nc()
    return _cached["nc"]


def _host_prep(hidden_states, fc_w, fc_b, start_transitions, transitions):
    """Build the 8 per-core input maps."""
    import ml_dtypes
    np_mdt = ml_dtypes.float8_e4m3

    E = np.exp(transitions.astype(np.float64)).astype(np.float32)     # [T,T]
    # epat[_row(g)+j,(c3,b,i)] = E[i,j], except chunk 0 (g=0, c3=0) slots =
    # exp(start_j + sigma) so the step-0 multiply by F = exp(e + fcb - sigma)
    # lands exactly on alpha_0 = exp(start + fcb + e_0) with no fixup op;
    # junk rows stay 0 so junk lanes propagate exact zeros
    epat = np.zeros((P_SCAN, C3, NB, T), dtype=np.float32)
    blk = np.tile(E.T[:, None, None, :], (1, C3, NB, 1))              # [j,c3,b,i]
    for g in range(NG):
        epat[_row(g):_row(g) + T] = blk
    epat[0:T, 0, :, :] = np.exp(
        start_transitions.astype(np.float64) + SIGMA)[:, None, None]
    epat = np.ascontiguousarray(epat.reshape(P_SCAN, NFREE))
    # lhsE: lhsT[_row(g)+k, _row(g)+j] = E[k,j]  (bf16 scan matmul)
    lhsE = np.zeros((P_SCAN, P_SCAN), dtype=ml_dtypes.bfloat16)
    Eb = E.astype(ml_dtypes.bfloat16)
    for g in range(NG):
        r = _row(g)
        lhsE[r:r + T, r:r + T] = Eb
    # fc weights in the DoubleRow SBUF image [128, ktp, two, 16], 9->16 pad
    fcwp = np.zeros((128, KT // 2, 2, 16), dtype=np_mdt)
    w = fc_w.astype(np_mdt)                                           # [T, H]
    for ktp in range(KT // 2):
        for two in range(2):
            kt = 2 * ktp + two
            fcwp[:, ktp, two, 0:T] = w[:, kt * 128:(kt + 1) * 128].T
    biasF = np.ascontiguousarray(
        (fc_b - SIGMA).reshape(T, 1), dtype=np.float32)

    in_maps = []
    for cid in range(NCORES):
        hc = hidden_states[cid * NB:(cid + 1) * NB]                   # [NB,S,H]
        # t-major token order: col = t*NB + b
        hc = hc.transpose(1, 0, 2).reshape(NTOK, H)
        hT6 = np.ascontiguousarray(hc.T.astype(np_mdt)).reshape(KT, 128, NTOK)
        hTq = np.empty((128, KT * NTOK), dtype=np_mdt)
        off = 0
        for banks in SLOT_BANKS:
            for ktp in range(KT // 2):
                for two in range(2):
                    kt = 2 * ktp + two
                    for g in banks:
                        hTq[:, off:off + 512] = hT6[kt, :, 512 * g:512 * g + 512]
                        off += 512
        in_maps.append({
            "hTq": hTq, "fcwp": fcwp, "lhsE": lhsE, "epat": epat,
            "biasF": biasF,
        })
    return in_maps


def _host_finish(results, hidden_states, labels, fc_w, fc_b,
                 start_transitions, end_transitions, transitions):
    """Numerator from raw inputs + chunk-matrix combine, all in f64."""
    labels = labels.astype(np.int64)
    start = start_transitions.astype(np.float64)
    end = end_transitions.astype(np.float64)
    trans = transitions.astype(np.float64)

    # numerator (mask all-ones fast path): emit[b,t] = h[b,t] . fc_w[l[b,t]]
    wg = fc_w[labels]                                     # [B,S,H] f32
    emit = np.einsum('bsh,bsh->bs', hidden_states, wg,
                     dtype=np.float64) + fc_b[labels].astype(np.float64)
    num = start[labels[:, 0]] + emit[:, 0]
    num = num + (trans[labels[:, :-1], labels[:, 1:]] + emit[:, 1:]).sum(1)
    num = num + end[labels[:, -1]]

    # denominator: combine chunk matrices.  q_out[_row(g)+j, (c3,b,i)] =
    # P_c[i,j] with c = 8g + c3.  Vectorize the chain over all sequences.
    P = np.empty((B, C, T, T))                            # P[b,c,i,j]
    for cid in range(NCORES):
        Q = results[cid]["q_out"].astype(np.float64)      # [105, 576]
        Qr = Q.reshape(P_OUT, C3, NB, T)                  # [p, c3, b, i]
        for g in range(NG):
            r = _row(g)
            # [j, c3, b, i] -> [b, c3, i, j]
            P[cid * NB:(cid + 1) * NB, 8 * g:8 * g + 8] = \
                Qr[r:r + T].transpose(2, 1, 3, 0)
    alpha = P[:, 0, 0, :].copy()          # rows of P_0 all equal alpha_0
    corr = np.zeros(B)
    for c in range(1, C):
        alpha = np.einsum('bi,bij->bj', alpha, P[:, c])
        m = alpha.max(axis=1)
        alpha /= m[:, None]
        corr += np.log(m)
    den = np.log((alpha * np.exp(end)[None, :]).sum(1)) + corr + (S - 1) * SIGMA
    return np.float32(-(num - den).sum())


def kernel(**inputs):
    hidden_states = np.asarray(inputs["hidden_states"], dtype=np.float32)
    attention_mask = np.asarray(inputs["attention_mask"])
    labels = np.asarray(inputs["labels"])
    fc_w = np.asarray(inputs["fc_w"], dtype=np.float32)
    fc_b = np.asarray(inputs["fc_b"], dtype=np.float32)
    start_transitions = np.asarray(inputs["start_transitions"], dtype=np.float32)
    end_transitions = np.asarray(inputs["end_transitions"], dtype=np.float32)
    transitions = np.asarray(inputs["transitions"], dtype=np.float32)

    if (hidden_states.shape != (B, S, H)) or not np.all(attention_mask != 0):
        return _reference_host(hidden_states, attention_mask, labels, fc_w,
                               fc_b, start_transitions, end_transitions,
                               transitions)

    from concourse.bass_utils import run_bass_kernel_spmd
    nc = _get_nc()
    in_maps = _host_prep(hidden_states, fc_w, fc_b, start_transitions,
                         transitions)
    res = run_bass_kernel_spmd(nc, in_maps, core_ids=list(range(NCORES)))
    _cached["last_res"] = res
    return _host_finish(res.results, hidden_states, labels, fc_w, fc_b,
                        start_transitions, end_transitions, transitions)
